# revision 2
# baseline (speedup 1.0000x reference)
"""GQA kernel for Trainium2, tensor-parallel over 8 NeuronCores.

Problem: B=2, S=2048, DIM=2048, 32 q-heads, 8 kv-heads, head_dim=64.
Sharding: core i owns kv-head i and q-heads 4i..4i+3 (Wq/Wk/Wv output-dim
sharded, Wo input-dim sharded). Each core computes a full [B*S,DIM] partial
of the output.

I/O strategy (minimizes host<->device bytes): x is uploaded token-sharded
(each core gets 512 tokens, 2MB bf16) and AllGathered on-device; the 8
partial outputs are ReduceScattered on-device so each core downloads only
its 512-token slice of the final output.

Per-core dataflow (all matmul operands bf16, fp32 PSUM accumulation):
  xT (host-pretransposed, [DIM, 512] shard) --> AllGather --> xg;
  QT/KT/VT projections with head-dim on partitions; scores computed
  transposed (S^T[k,q] = KT_blk^T @ QT), exp on ScalarE with fused
  1/sqrt(hd) scale (max-subtraction skipped: scores are N(0,1)-bounded);
  AV uses lhsT=[V | 1] so the softmax denominator lands in PSUM row 64;
  normalization via reciprocal + rank-1 broadcast matmul; O-proj consumes
  the attention output directly in its [dq, tok] layout; partials go to an
  internal DRAM buffer that feeds the ReduceScatter.
"""
import sys

import numpy as np

sys.path.insert(0, "/opt/trn_rl_repo")

import ml_dtypes
import concourse.bacc as bacc
import concourse.tile as tile
from concourse import mybir
from concourse.masks import make_identity
from concourse import bass_utils

F32 = mybir.dt.float32
BF16 = mybir.dt.bfloat16

B, S, DIM = 2, 2048, 2048
N_HEADS, N_KV = 32, 8
HD = DIM // N_HEADS          # 64
G = N_HEADS // N_KV          # 4 q-heads per kv head (= per core)
DQ = G * HD                  # 256 q-proj cols per core
NCORES = 8
TOKS = B * S                 # 4096
SHARD = TOKS // NCORES       # 512 tokens uploaded per core
CT = DIM // 128              # 16 contraction tiles
TT = S // 512                # 4 tok-tiles of 512 per batch
KT_N = S // 128              # 16 key tiles of 128 per batch
SM_SCALE = HD ** -0.5

_CACHE = {}


def _build():
    nc = bacc.Bacc("TRN2", debug=False, num_devices=NCORES)

    xs = nc.dram_tensor("xs", [DIM, SHARD], BF16, kind="ExternalInput")
    wq = nc.dram_tensor("wq", [DIM, DQ], BF16, kind="ExternalInput")
    wkv = nc.dram_tensor("wkv", [DIM, 2 * HD], BF16, kind="ExternalInput")
    wo = nc.dram_tensor("wo", [DQ, DIM], BF16, kind="ExternalInput")
    out_p = nc.dram_tensor("out_p", [SHARD, DIM], BF16, kind="ExternalOutput")

    with tile.TileContext(nc) as tc:
        with (
            tc.tile_pool(name="dram", bufs=1, space="DRAM") as dram,
            tc.tile_pool(name="wpool", bufs=1) as wpool,
            tc.tile_pool(name="xpool", bufs=2) as xpool,
            tc.tile_pool(name="actp", bufs=1) as actp,
            tc.tile_pool(name="epool", bufs=3) as epool,
            tc.tile_pool(name="small", bufs=4) as small,
            tc.tile_pool(name="pps", bufs=1, space="PSUM") as pps,
        ):
            # ---- device-side gather of the token-sharded input ----
            ag_in = dram.tile([DIM, SHARD], BF16)
            xg = dram.tile([NCORES * DIM, SHARD], BF16, addr_space="Shared")
            nc.gpsimd.dma_start(ag_in[:], xs.ap())
            nc.gpsimd.collective_compute(
                "AllGather",
                mybir.AluOpType.bypass,
                replica_groups=[list(range(NCORES))],
                ins=[ag_in.opt()],
                outs=[xg.opt()],
            )
            out_part = dram.tile([TOKS, DIM], BF16)

            # ---- stage weights ----
            wq_sb = wpool.tile([128, CT, 2, 128], BF16)
            nc.scalar.dma_start(
                wq_sb[:], wq.ap().rearrange("(ct p) (dt m) -> p ct dt m", p=128, m=128)
            )
            wkv_sb = wpool.tile([128, CT, 128], BF16)
            nc.scalar.dma_start(
                wkv_sb[:], wkv.ap().rearrange("(ct p) d -> p ct d", p=128)
            )
            wo_sb = wpool.tile([128, 2, 4, 512], BF16)
            nc.scalar.dma_start(
                wo_sb[:], wo.ap().rearrange("(dt p) (nt n) -> p dt nt n", p=128, n=512)
            )
            ident = wpool.tile([64, 64], BF16)
            make_identity(nc, ident[:])
            ones64 = wpool.tile([1, 64], BF16)
            nc.vector.memset(ones64[:], 1.0)

            for b in range(B):
                # ---- projections: QT[dq,tok], KT[dk,tok], VT[dv,tok] ----
                qt_g = [actp.tile([64, S], BF16, tag=f"qt{g}", name=f"qt{g}", bufs=2) for g in range(G)]
                kt = actp.tile([64, S], BF16, tag="kt", bufs=2)
                vt = actp.tile([64, S], BF16, tag="vt", bufs=2)
                v1 = actp.tile([128, KT_N, 65], BF16, tag="v1", bufs=2)
                ao2 = [actp.tile([128, S], BF16, tag=f"ao{d}", name=f"ao{d}", bufs=2) for d in range(2)]

                for tt in range(TT):
                    slab = b * TT + tt
                    xc = xpool.tile([128, CT, 512], BF16, tag="xc")
                    dma_eng = nc.sync if tt % 2 == 0 else nc.gpsimd
                    dma_eng.dma_start(
                        xc[:],
                        xg[slab * DIM:(slab + 1) * DIM, :]
                        .rearrange("(ct p) n -> p ct n", p=128),
                    )
                    psum_q = pps.tile([128, 2, 512], F32, tag="big2", bufs=2)
                    psum_kv = pps.tile([128, 512], F32, tag="one", bufs=4)
                    for ci in range(CT):
                        st, sp = ci == 0, ci == CT - 1
                        for dt in range(2):
                            nc.tensor.matmul(psum_q[:, dt, :], wq_sb[:, ci, dt, :],
                                             xc[:, ci, :], start=st, stop=sp)
                        nc.tensor.matmul(psum_kv[:], wkv_sb[:, ci, :],
                                         xc[:, ci, :], start=st, stop=sp)
                    qs_ = slice(tt * 512, (tt + 1) * 512)
                    for g in range(G):
                        nc.vector.tensor_copy(
                            qt_g[g][:, qs_],
                            psum_q[:, g // 2, :][(g % 2) * 64:(g % 2) * 64 + 64, :],
                        )
                    nc.vector.tensor_copy(kt[:, qs_], psum_kv[0:64, :])
                    nc.vector.tensor_copy(vt[:, qs_], psum_kv[64:128, :])

                # ---- V natural [tok,dv] + ones column ----
                nc.vector.memset(v1[:, :, 64:65], 1.0)
                for ki in range(KT_N):
                    p_tr = pps.tile([128, 512], BF16, tag="one", bufs=4, name="p_tr")
                    nc.tensor.transpose(p_tr[:, 0:64], vt[:, ki * 128:(ki + 1) * 128],
                                        ident[:])
                    nc.vector.tensor_copy(v1[:, ki, 0:64], p_tr[:, 0:64])

                # ---- attention per q-head, split into two q-halves ----
                for g2 in range(2 * G):
                    g, qh = g2 // 2, g2 % 2
                    av = [pps.tile([128, 512], F32, tag="one", bufs=4, name=f"av{qs}") for qs in range(2)]
                    for ki in range(KT_N):
                        st, sp = ki == 0, ki == KT_N - 1
                        ps_s = pps.tile([128, 2, 512], F32, tag="big2", bufs=2, name="ps_s")
                        for qs in range(2):
                            nc.tensor.matmul(
                                ps_s[:, qs, :],
                                kt[:, ki * 128:(ki + 1) * 128],
                                qt_g[g][:, (qh * 2 + qs) * 512:(qh * 2 + qs + 1) * 512],
                                start=True, stop=True,
                            )
                        e_sb = epool.tile([128, 1024], BF16, tag="e", bufs=6)
                        nc.scalar.activation(e_sb[:], ps_s[:],
                                             mybir.ActivationFunctionType.Exp,
                                             scale=SM_SCALE)
                        for qs in range(2):
                            nc.tensor.matmul(
                                av[qs][0:65, :], v1[:, ki, :],
                                e_sb[:, qs * 512:(qs + 1) * 512],
                                start=st, stop=sp,
                            )
                    for qs2 in range(2):
                        qt = qh * 2 + qs2
                        raw = small.tile([65, 512], F32, tag="raw", bufs=2)
                        nc.vector.tensor_copy(raw[:], av[qs2][0:65, :])
                        den = small.tile([1, 512], F32, tag="den")
                        nc.vector.tensor_copy(den[:], raw[64:65, :])
                        nc.vector.reciprocal(den[:], den[:])
                        den_b = small.tile([1, 512], BF16, tag="denb")
                        nc.vector.tensor_copy(den_b[:], den[:])
                        p_bc = pps.tile([128, 512], F32, tag="one", bufs=4, name="p_bc")
                        nc.tensor.matmul(p_bc[0:64, :], ones64[:], den_b[:],
                                         start=True, stop=True)
                        bc_sb = small.tile([64, 512], F32, tag="bc")
                        nc.vector.tensor_copy(bc_sb[:], p_bc[0:64, :])
                        nc.vector.tensor_mul(
                            ao2[g // 2][(g % 2) * 64:(g % 2) * 64 + 64,
                                        qt * 512:(qt + 1) * 512],
                            raw[0:64, :], bc_sb[:],
                        )

                # ---- O-projection ----
                for t2 in range(S // 128):
                    o_sb = epool.tile([128, 4, 512], BF16, tag="osb", bufs=3)
                    for half in range(2):
                        po = pps.tile([128, 2, 512], F32, tag="big2", bufs=2,
                                      name="po")
                        for dt in range(2):
                            for nt in range(2):
                                nc.tensor.matmul(
                                    po[:, nt, :],
                                    ao2[dt][:, t2 * 128:(t2 + 1) * 128],
                                    wo_sb[:, dt, half * 2 + nt, :],
                                    start=dt == 0, stop=dt == 1,
                                )
                        nc.vector.tensor_copy(
                            o_sb[:, half * 2:(half + 1) * 2, :], po[:])
                    out_eng = (nc.sync, nc.gpsimd, nc.scalar)[t2 % 3]
                    out_eng.dma_start(
                        out_part[b * S + t2 * 128:b * S + (t2 + 1) * 128, :], o_sb[:]
                    )

            # ---- device-side reduce of the partial outputs ----
            rs_out = dram.tile([SHARD, DIM], BF16)
            nc.gpsimd.collective_compute(
                "ReduceScatter",
                mybir.AluOpType.add,
                replica_groups=[list(range(NCORES))],
                ins=[out_part.opt()],
                outs=[rs_out.opt()],
            )
            nc.sync.dma_start(out_p.ap(), rs_out[:])

    nc.compile()
    return nc


def _get_nc():
    if "nc" not in _CACHE:
        _CACHE["nc"] = _build()
    return _CACHE["nc"]


def kernel(x, Wq, Wk, Wv, Wo, _trace=False):
    nc = _get_nc()
    bf = ml_dtypes.bfloat16
    xT = np.ascontiguousarray(
        np.asarray(x, np.float32).transpose(2, 0, 1).reshape(DIM, TOKS)
    ).astype(bf)
    Wq = np.asarray(Wq, np.float32)
    Wk = np.asarray(Wk, np.float32)
    Wv = np.asarray(Wv, np.float32)
    Wo = np.asarray(Wo, np.float32)

    in_maps = []
    for c in range(NCORES):
        wq_c = Wq[:, c * DQ:(c + 1) * DQ].astype(bf)
        wkv_c = np.concatenate(
            [Wk[:, c * HD:(c + 1) * HD], Wv[:, c * HD:(c + 1) * HD]], axis=1
        ).astype(bf)
        wo_c = Wo[c * DQ:(c + 1) * DQ, :].astype(bf)
        in_maps.append({
            "xs": np.ascontiguousarray(xT[:, c * SHARD:(c + 1) * SHARD]),
            "wq": np.ascontiguousarray(wq_c),
            "wkv": np.ascontiguousarray(wkv_c),
            "wo": np.ascontiguousarray(wo_c),
        })

    res = bass_utils.run_bass_kernel_spmd(
        nc, in_maps, core_ids=list(range(NCORES)), trace=_trace
    )
    out = np.concatenate(
        [res.results[c]["out_p"].astype(np.float32) for c in range(NCORES)], axis=0
    )
    if _trace:
        kernel.last_exec_time_ns = res.exec_time_ns
        kernel.last_results = res
    return out.reshape(B, S, DIM)


kernel.last_exec_time_ns = None


# revision 8
# speedup vs baseline: 1.1137x; 1.1137x over previous
"""GQA kernel for Trainium2, tensor-parallel over 8 NeuronCores.

Problem: B=2, S=2048, DIM=2048, 32 q-heads, 8 kv-heads, head_dim=64.
Sharding: core i owns kv-head i and q-heads 4i..4i+3 (Wq/Wk/Wv output-dim
sharded, Wo input-dim sharded). Each core computes a full [B*S,DIM] partial
of the output.

I/O strategy (minimizes host<->device bytes): x is uploaded token-sharded
(each core gets 512 tokens, 2MB bf16) and AllGathered on-device in two
chunks (so projections start after the first); the per-batch partial
outputs are ReduceScattered on-device (batch 0's reduce overlaps batch 1's
compute) so each core downloads only its slice of the final output.

Per-core dataflow (all matmul operands bf16, fp32 PSUM accumulation):
  QT/KT are built with the head-dim duplicated into both PE row-halves so
  score matmuls (contraction=64) pack two key-blocks concurrently via PE
  tile_position; exp on ScalarE with fused 1/sqrt(hd) scale
  (max-subtraction skipped: scores are N(0,1)-bounded); AV uses
  lhsT=[V | 1] so the softmax denominator lands in PSUM row 64;
  normalization via reciprocal_approx_fast + rank-1 broadcast matmul, with
  the element-wise ops reading PSUM directly; O-proj consumes the
  attention output in its [dq, tok] layout.
"""
import sys

import numpy as np

sys.path.insert(0, "/opt/trn_rl_repo")

import ml_dtypes
import concourse.bacc as bacc
import concourse.tile as tile
from concourse import mybir
from concourse.masks import make_identity
from concourse import bass_utils

F32 = mybir.dt.float32
BF16 = mybir.dt.bfloat16

B, S, DIM = 2, 2048, 2048
N_HEADS, N_KV = 32, 8
HD = DIM // N_HEADS          # 64
G = N_HEADS // N_KV          # 4 q-heads per kv head (= per core)
DQ = G * HD                  # 256 q-proj cols per core
NCORES = 8
TOKS = B * S                 # 4096
SHARD = TOKS // NCORES       # 512 tokens uploaded per core
RSH = SHARD // B             # 256 output rows per core per batch
CT = DIM // 128              # 16 contraction tiles
TT = S // 512                # 4 tok-tiles of 512 per batch
KT_N = S // 128              # 16 key tiles of 128 per batch
KP_N = KT_N // 2             # 8 key-tile pairs
SM_SCALE = HD ** -0.5

_CACHE = {}


def _build():
    nc = bacc.Bacc("TRN2", debug=False, num_devices=NCORES)

    xs = nc.dram_tensor("xs", [DIM, SHARD], BF16, kind="ExternalInput")
    wq = nc.dram_tensor("wq", [DIM, DQ], BF16, kind="ExternalInput")
    wkv = nc.dram_tensor("wkv", [DIM, 2 * HD], BF16, kind="ExternalInput")
    wo = nc.dram_tensor("wo", [DQ, DIM], BF16, kind="ExternalInput")
    out_p = nc.dram_tensor("out_p", [B * RSH, DIM], BF16, kind="ExternalOutput")

    with tile.TileContext(nc) as tc:
        with (
            tc.tile_pool(name="dram", bufs=1, space="DRAM") as dram,
            tc.tile_pool(name="wpool", bufs=1) as wpool,
            tc.tile_pool(name="xpool", bufs=2) as xpool,
            tc.tile_pool(name="actp", bufs=1) as actp,
            tc.tile_pool(name="epool", bufs=3) as epool,
            tc.tile_pool(name="small", bufs=4) as small,
            tc.tile_pool(name="pps", bufs=1, space="PSUM") as pps,
        ):
            # ---- device-side gather of the token-sharded input (2 chunks) ----
            HCT = CT // 2  # contraction tiles per AG chunk
            HD2 = DIM // 2
            ag_in = [dram.tile([HD2, SHARD], BF16, name=f"agi{h}") for h in range(2)]
            xg = [
                dram.tile([NCORES * HD2, SHARD], BF16, addr_space="Shared",
                          name=f"xg{h}")
                for h in range(2)
            ]
            for h in range(2):
                nc.gpsimd.dma_start(ag_in[h][:], xs.ap()[h * HD2:(h + 1) * HD2, :])
                nc.gpsimd.collective_compute(
                    "AllGather",
                    mybir.AluOpType.bypass,
                    replica_groups=[list(range(NCORES))],
                    ins=[ag_in[h].opt()],
                    outs=[xg[h].opt()],
                )
            out_part = [dram.tile([S, DIM], BF16, name=f"opart{b}") for b in range(B)]

            # ---- stage weights ----
            wq_sb = wpool.tile([128, CT, 2, 128], BF16)
            nc.scalar.dma_start(
                wq_sb[:], wq.ap().rearrange("(ct p) (dt m) -> p ct dt m", p=128, m=128)
            )
            wkv_sb = wpool.tile([128, CT, 128], BF16)
            nc.scalar.dma_start(
                wkv_sb[:], wkv.ap().rearrange("(ct p) d -> p ct d", p=128)
            )
            wo_sb = wpool.tile([128, 2, 4, 512], BF16)
            nc.scalar.dma_start(
                wo_sb[:], wo.ap().rearrange("(dt p) (nt n) -> p dt nt n", p=128, n=512)
            )
            ident = wpool.tile([64, 64], BF16)
            make_identity(nc, ident[:])
            ones64 = wpool.tile([1, 64], BF16)
            nc.vector.memset(ones64[:], 1.0)

            for b in range(B):
                # ---- projections: QT/KT with head-dim duplicated into both
                # PE row-halves (for packed score matmuls), VT natural ----
                qt_g = [actp.tile([128, S], BF16, tag=f"qt{g}", name=f"qt{g}", bufs=2) for g in range(G)]
                kt = actp.tile([128, S], BF16, tag="kt", bufs=2)
                vt = actp.tile([64, S], BF16, tag="vt", bufs=2)
                v1 = actp.tile([128, KT_N, 65], BF16, tag="v1", bufs=2)
                ao2 = [actp.tile([128, S], BF16, tag=f"ao{d}", name=f"ao{d}", bufs=2) for d in range(2)]

                for tt in range(TT):
                    slab = b * TT + tt
                    xc = [xpool.tile([128, HCT, 512], BF16, tag=f"xc{h}",
                                     name=f"xc{h}") for h in range(2)]
                    for h in range(2):
                        dma_eng = nc.sync if (tt + h) % 2 == 0 else nc.gpsimd
                        dma_eng.dma_start(
                            xc[h][:],
                            xg[h][slab * HD2:(slab + 1) * HD2, :]
                            .rearrange("(ct p) n -> p ct n", p=128),
                        )
                    psum_q = pps.tile([128, 2, 512], F32, tag="sc", bufs=2)
                    psum_kv = pps.tile([128, 512], F32, tag="one", bufs=4)
                    for ci in range(CT):
                        st, sp = ci == 0, ci == CT - 1
                        xcs = xc[ci // HCT][:, ci % HCT, :]
                        for dt in range(2):
                            nc.tensor.matmul(psum_q[:, dt, :], wq_sb[:, ci, dt, :],
                                             xcs, start=st, stop=sp)
                        nc.tensor.matmul(psum_kv[:], wkv_sb[:, ci, :],
                                         xcs, start=st, stop=sp)
                    qs_ = slice(tt * 512, (tt + 1) * 512)
                    # duplicate each head's 64 rows into both row-halves;
                    # spread copies across engines (ACT is idle here)
                    for g in range(G):
                        src = psum_q[:, g // 2, :][(g % 2) * 64:(g % 2) * 64 + 64, :]
                        nc.vector.tensor_copy(qt_g[g][0:64, qs_], src)
                        nc.scalar.copy(qt_g[g][64:128, qs_], src)
                    nc.vector.tensor_copy(kt[0:64, qs_], psum_kv[0:64, :])
                    nc.scalar.copy(kt[64:128, qs_], psum_kv[0:64, :])
                    nc.vector.tensor_copy(vt[:, qs_], psum_kv[64:128, :])

                # ---- V natural [tok,dv] + ones column ----
                nc.vector.memset(v1[:, :, 64:65], 1.0)
                for ki in range(KT_N):
                    p_tr = pps.tile([128, 512], BF16, tag="one", bufs=4, name="p_tr")
                    nc.tensor.transpose(p_tr[:, 0:64], vt[:, ki * 128:(ki + 1) * 128],
                                        ident[:])
                    if ki % 2 == 0:
                        nc.vector.tensor_copy(v1[:, ki, 0:64], p_tr[:, 0:64])
                    else:
                        nc.scalar.copy(v1[:, ki, 0:64], p_tr[:, 0:64])

                # ---- attention per q-head, two tok-halves of 1024 ----
                for g2 in range(2 * G):
                    g, qh = g2 // 2, g2 % 2
                    av = [pps.tile([128, 512], F32, tag="one", bufs=4, name=f"av{qs}") for qs in range(2)]
                    for kp in range(KP_N):
                        kiA, kiB = 2 * kp, 2 * kp + 1
                        ps_2 = [pps.tile([128, 2, 512], F32, tag="sc", bufs=2,
                                         name=f"ps{h}") for h in range(2)]
                        for qs in range(2):
                            q_sl = slice((qh * 2 + qs) * 512, (qh * 2 + qs + 1) * 512)
                            # kiA on PE rows 0-63, kiB on rows 64-127: the two
                            # 64-contraction matmuls run concurrently
                            nc.tensor.matmul(
                                ps_2[0][:, qs, :],
                                kt[0:64, kiA * 128:(kiA + 1) * 128],
                                qt_g[g][0:64, q_sl], start=True, stop=True,
                            )
                            nc.tensor.matmul(
                                ps_2[1][:, qs, :],
                                kt[64:128, kiB * 128:(kiB + 1) * 128],
                                qt_g[g][64:128, q_sl], start=True, stop=True,
                            )
                        e_sb = epool.tile([128, 2, 1024], BF16, tag="e", bufs=3)
                        for h in range(2):
                            nc.scalar.activation(e_sb[:, h, :], ps_2[h][:],
                                                 mybir.ActivationFunctionType.Exp,
                                                 scale=SM_SCALE)
                        for qs in range(2):
                            for h, ki in ((0, kiA), (1, kiB)):
                                nc.tensor.matmul(
                                    av[qs][0:65, :], v1[:, ki, :],
                                    e_sb[:, h, qs * 512:(qs + 1) * 512],
                                    start=(kp == 0 and h == 0),
                                    stop=(kp == KP_N - 1 and h == 1),
                                )
                    for qs2 in range(2):
                        qt = qh * 2 + qs2
                        den_s = small.tile([1, 512], F32, tag="dens")
                        nc.vector.tensor_copy(den_s[:], av[qs2][64:65, :])
                        den = small.tile([1, 512], F32, tag="den")
                        nc.vector.reciprocal_approx_fast(den[:], den_s[:])
                        den_b = small.tile([1, 512], BF16, tag="denb")
                        nc.vector.tensor_copy(den_b[:], den[:])
                        p_bc = pps.tile([128, 512], F32, tag="one", bufs=4, name="p_bc")
                        nc.tensor.matmul(p_bc[0:64, :], ones64[:], den_b[:],
                                         start=True, stop=True)
                        bc_sb = small.tile([64, 512], F32, tag="bc")
                        nc.vector.tensor_copy(bc_sb[:], p_bc[0:64, :])
                        nc.vector.tensor_mul(
                            ao2[g // 2][(g % 2) * 64:(g % 2) * 64 + 64,
                                        qt * 512:(qt + 1) * 512],
                            av[qs2][0:64, :], bc_sb[:],
                        )

                # ---- O-projection ----
                for t2 in range(S // 128):
                    o_sb = epool.tile([128, 4, 512], BF16, tag="osb", bufs=3)
                    for half in range(2):
                        po = pps.tile([128, 2, 512], F32, tag="sc", bufs=2,
                                      name="po")
                        for dt in range(2):
                            for nt in range(2):
                                nc.tensor.matmul(
                                    po[:, nt, :],
                                    ao2[dt][:, t2 * 128:(t2 + 1) * 128],
                                    wo_sb[:, dt, half * 2 + nt, :],
                                    start=dt == 0, stop=dt == 1,
                                )
                        if half == 0:
                            nc.vector.tensor_copy(o_sb[:, 0:2, :], po[:])
                        else:
                            nc.scalar.copy(o_sb[:, 2:4, :], po[:])
                    out_eng = (nc.sync, nc.gpsimd, nc.scalar)[t2 % 3]
                    out_eng.dma_start(
                        out_part[b][t2 * 128:(t2 + 1) * 128, :], o_sb[:]
                    )

                # ---- per-batch device-side reduce (b0's overlaps b1 compute) ----
                rs_out = dram.tile([RSH, DIM], BF16, name=f"rso{b}")
                nc.gpsimd.collective_compute(
                    "ReduceScatter",
                    mybir.AluOpType.add,
                    replica_groups=[list(range(NCORES))],
                    ins=[out_part[b].opt()],
                    outs=[rs_out.opt()],
                )
                nc.sync.dma_start(
                    out_p.ap()[b * RSH:(b + 1) * RSH, :], rs_out[:]
                )

    nc.compile()
    return nc


def _get_nc():
    if "nc" not in _CACHE:
        _CACHE["nc"] = _build()
    return _CACHE["nc"]


def kernel(x, Wq, Wk, Wv, Wo, _trace=False):
    nc = _get_nc()
    bf = ml_dtypes.bfloat16
    xT = np.ascontiguousarray(
        np.asarray(x, np.float32).transpose(2, 0, 1).reshape(DIM, TOKS)
    ).astype(bf)
    Wq = np.asarray(Wq, np.float32)
    Wk = np.asarray(Wk, np.float32)
    Wv = np.asarray(Wv, np.float32)
    Wo = np.asarray(Wo, np.float32)

    in_maps = []
    for c in range(NCORES):
        wq_c = Wq[:, c * DQ:(c + 1) * DQ].astype(bf)
        wkv_c = np.concatenate(
            [Wk[:, c * HD:(c + 1) * HD], Wv[:, c * HD:(c + 1) * HD]], axis=1
        ).astype(bf)
        wo_c = Wo[c * DQ:(c + 1) * DQ, :].astype(bf)
        in_maps.append({
            "xs": np.ascontiguousarray(xT[:, c * SHARD:(c + 1) * SHARD]),
            "wq": np.ascontiguousarray(wq_c),
            "wkv": np.ascontiguousarray(wkv_c),
            "wo": np.ascontiguousarray(wo_c),
        })

    res = bass_utils.run_bass_kernel_spmd(
        nc, in_maps, core_ids=list(range(NCORES)), trace=_trace
    )
    # core c's out_p rows: [b*RSH:(b+1)*RSH] = tokens b*S + c*RSH + (0..RSH)
    out = np.empty((TOKS, DIM), np.float32)
    for c in range(NCORES):
        o = res.results[c]["out_p"].astype(np.float32)
        for b in range(B):
            out[b * S + c * RSH:b * S + (c + 1) * RSH] = o[b * RSH:(b + 1) * RSH]
    if _trace:
        kernel.last_exec_time_ns = res.exec_time_ns
        kernel.last_results = res
    return out.reshape(B, S, DIM)


kernel.last_exec_time_ns = None
